# revision 18
# baseline (speedup 1.0000x reference)
"""Trainium2 Bass kernel for nn_DifferentiableFeatureExtractor.

Strategy (8 NeuronCores, shard T=1048576 along time):
  - per-core extended domain EXT = S + 2048 halo = 133120 = 128 partitions x 1040
  - each partition holds a contiguous 1072-bar chunk plus a 256-bar AP halo
    (tile [128, 1328]); host supplies a 256-bar lead-in so partition 0's halo
    is real data (clamp-padded at the global left edge like the reference)
  - 20 truncated-EMA convs as fp32 recurrences: y = a*(s - c^K s[t-K]) with
    s from tensor_tensor_scan (2-pass blocked scan, partition carry chained
    via shifted end-columns; c^CH second-order term kept when significant)
  - sliding max/min via log-doubling with shifted APs (halo-local)
  - rolling std via anchored windowed sums (doubling)
  - BARSLAST/MA_DYNAMIC as segmented scans (reset at cross events) with
    affine partition-carry chains; exact whenever the previous cross lies
    within the 2048-bar halo (diag output flags violations -> host fallback)
All heavy compute runs on device; host only shards, gathers, patches the
17 reference partial-window std bars, and handles the (never-observed)
diag fallback.
"""
import math

import numpy as np

import concourse.bacc as bacc
from concourse.bass_types import AP as BassAP
import concourse.mybir as mybir
from concourse import tile as ctile
from concourse.bass_utils import run_bass_kernel_spmd

F32 = mybir.dt.float32
Alu = mybir.AluOpType
Act = mybir.ActivationFunctionType

T = 1048576
NCORES = 8
S = T // NCORES            # 131072
P = 128
CH = 1040                  # chunk cols per partition
HP = 256                   # per-partition halo cols
W = HP + CH                # 1328
EXT = P * CH               # 133120
HALO = EXT - S             # 2048
DLEN = HP + EXT            # 133376
C0 = HP                    # chunk start col
NROWS = 30

# static truncation lengths (depend only on the reference's constant ALPHAS)
KS = [72, 72, 72, 286, 286, 286, 559, 89, 54, 47, 40, 30, 130, 30,
      30, 30, 30, 37, 37, 37]


class KB:
    """kernel builder with a simple big-tile freelist"""

    def __init__(self, alphas, anchor):
        self.alphas = [float(a) for a in alphas]
        self.anchor = float(anchor)
        nc = bacc.Bacc(None, target_bir_lowering=False)
        self.nc = nc
        self.DC = nc.dram_tensor("DC", [DLEN], F32, kind="ExternalInput")
        self.DH = nc.dram_tensor("DH", [DLEN], F32, kind="ExternalInput")
        self.DL = nc.dram_tensor("DL", [DLEN], F32, kind="ExternalInput")
        self.OUT = nc.dram_tensor("OUT", [NROWS * EXT], F32, kind="ExternalOutput")
        self.DIAG = nc.dram_tensor("DIAG", [2], F32, kind="ExternalOutput")
        self.free_big = []
        self.n_big = 0
        self.free_small = []
        self.n_small = 0
        self.free_row = []
        self.n_row = 0
        self.free_row129 = []
        self.n_row129 = 0

    # ---- tile management ----
    def big(self):
        if self.free_big:
            return self.free_big.pop(0)
        t = self.pool.tile([P, W], F32, tag=f"big{self.n_big}")
        self.n_big += 1
        return t

    def rel(self, *ts):
        for t in ts:
            self.free_big.append(t)

    def small(self):
        if self.free_small:
            return self.free_small.pop()
        t = self.spool.tile([P, 1], F32, tag=f"small{self.n_small}")
        self.n_small += 1
        return t

    def rels(self, *ts):
        for t in ts:
            self.free_small.append(t)

    def row(self):
        if self.free_row:
            return self.free_row.pop()
        t = self.spool.tile([1, P], F32, tag=f"row{self.n_row}")
        self.n_row += 1
        return t

    def relr(self, *ts):
        for t in ts:
            self.free_row.append(t)

    def row129(self):
        if self.free_row129:
            return self.free_row129.pop()
        t = self.spool.tile([1, P + 1], F32, tag=f"row129_{self.n_row129}")
        self.n_row129 += 1
        return t

    def relr129(self, *ts):
        for t in ts:
            self.free_row129.append(t)

    # ---- IO ----
    def load_series(self, dram, eng="sync"):
        nc = self.nc
        t = self.big()
        base = dram[0:DLEN].rearrange("(a b) -> a b", a=1, b=DLEN)
        src_ap = BassAP(base.tensor, 0, [[CH, P], [1, W]])
        getattr(nc, eng).dma_start(out=t[:, 0:W], in_=src_ap)
        return t

    def store_row(self, r, t):
        nc = self.nc
        nc.sync.dma_start(
            out=self.OUT[r * EXT : (r + 1) * EXT].rearrange(
                "(p w) -> p w", p=P, w=CH
            ),
            in_=t[:, C0:W],
        )

    # ---- building blocks ----
    def ema(self, xt, i, scale=1.0):
        """truncated EMA of xt (valid on chunk cols) -> new tile valid [248, W).
        Output scaled by `scale`."""
        nc = self.nc
        a = self.alphas[i]
        K = KS[i]
        c = 1.0 - a
        cF = float(c) ** CH
        cK = float(c) ** K

        s = self.big()
        # scan1: local scan over a truncation-sufficient tail window
        V = min(CH, 2 * K + 16) if i < 6 else min(CH, K + 64)
        cbc = self.CONSTS[:, i : i + 1].broadcast_to([P, V])
        nc.vector.tensor_tensor_scan(
            out=s[:, W - V : W], data0=cbc,
            data1=xt[:, W - V : W], initial=0.0, op0=Alu.mult, op1=Alu.add,
        )
        # partition carry via PE shift-matmul: carry[p] = E[p-1] (+cF*E[p-2])
        mmat = self.M2.get(i, self.Sh1)
        pcar = self.pscol.tile([P, 1], F32, tag="pscol")
        self.mm(pcar[:, 0:1], mmat[:, :], s[:, W - 1 : W])
        # scan2: chained scan with PSUM initial
        cbc2 = self.CONSTS[:, i : i + 1].broadcast_to([P, CH])
        nc.vector.tensor_tensor_scan(
            out=s[:, C0:W], data0=cbc2,
            data1=xt[:, C0:W], initial=pcar[:, 0:1], op0=Alu.mult, op1=Alu.add,
        )
        # halo values via PE one-partition shift of the chunk tail; the
        # corrections read them straight from PSUM (no halo-ACT pass)
        ph = self.pshalo.tile([P, HP], F32, tag="psh")
        self.mm(ph[:, 0:HP], self.Sh1[:, :], s[:, CH:W])
        ss = self.big()
        # ss chunk region + the 8 halo cols [248, HP) the corrections read
        nc.scalar.mul(ss[:, HP:W], s[:, C0:W], -a * scale)
        nc.scalar.mul(ss[:, 248:HP], ph[:, 248:HP], -a * scale)
        self.rel(s)
        y = self.big()
        nas = -a * scale
        if K <= 248:
            # head (cols [248, 256+K)): ss[t-K] lives in the halo -> PSUM ph
            nc.vector.scalar_tensor_tensor(
                out=y[:, 248 : 256 + K], in0=ph[:, 248 - K : 256],
                scalar=cK * nas, in1=ss[:, 248 : 256 + K],
                op0=Alu.mult, op1=Alu.subtract,
            )
            # main (cols [256+K, W)): chunk-only reads
            nc.vector.scalar_tensor_tensor(
                out=y[:, 256 + K : W], in0=ss[:, 256 : W - K], scalar=cK,
                in1=ss[:, 256 + K : W], op0=Alu.mult, op1=Alu.subtract,
            )
        else:
            # piece A (cols [248, K)): one-partition shift via PE (unscaled s
            # tail was consumed before rel; shift ss chunk instead)
            pa = self.pshalo.tile([P, K - 248], F32, tag="psh")
            self.mm(pa[:, 0 : K - 248], self.Sh1[:, :], ss[:, 248 + CH - K : CH])
            nc.vector.scalar_tensor_tensor(
                out=y[:, 248:K], in0=pa[:, 0 : K - 248], scalar=cK,
                in1=ss[:, 248:K], op0=Alu.mult, op1=Alu.subtract,
            )
            # piece B1 (cols [K, 256+K)): halo reads from PSUM ph
            nc.vector.scalar_tensor_tensor(
                out=y[:, K : 256 + K], in0=ph[:, 0:256], scalar=cK * nas,
                in1=ss[:, K : 256 + K], op0=Alu.mult, op1=Alu.subtract,
            )
            # piece B2 (cols [256+K, W)): same-partition chunk AP shift
            nc.vector.scalar_tensor_tensor(
                out=y[:, 256 + K : W], in0=ss[:, 256 : W - K], scalar=cK,
                in1=ss[:, 256 + K : W], op0=Alu.mult, op1=Alu.subtract,
            )
        self.rel(ss)
        return y

    def ema_front(self, xt, i):
        """scan stages of ema(); returns state for ema_finish."""
        nc = self.nc
        a = self.alphas[i]
        K = KS[i]
        s = self.big()
        V = min(CH, 2 * K + 16) if i < 6 else min(CH, K + 64)
        cbc = self.CONSTS[:, i : i + 1].broadcast_to([P, V])
        nc.vector.tensor_tensor_scan(
            out=s[:, W - V : W], data0=cbc,
            data1=xt[:, W - V : W], initial=0.0, op0=Alu.mult, op1=Alu.add,
        )
        mmat = self.M2.get(i, self.Sh1)
        pcar = self.pscol.tile([P, 1], F32, tag="pscol")
        self.mm(pcar[:, 0:1], mmat[:, :], s[:, W - 1 : W])
        cbc2 = self.CONSTS[:, i : i + 1].broadcast_to([P, CH])
        nc.vector.tensor_tensor_scan(
            out=s[:, C0:W], data0=cbc2,
            data1=xt[:, C0:W], initial=pcar[:, 0:1], op0=Alu.mult, op1=Alu.add,
        )
        return (s, i)

    def ema_finish(self, state, scale=1.0):
        """ACT scale + correction stages of ema()."""
        nc = self.nc
        s, i = state
        a = self.alphas[i]
        K = KS[i]
        c = 1.0 - a
        cK = float(c) ** K
        ph = self.pshalo.tile([P, HP], F32, tag="psh")
        self.mm(ph[:, 0:HP], self.Sh1[:, :], s[:, CH:W])
        ss = self.big()
        nc.scalar.mul(ss[:, HP:W], s[:, C0:W], -a * scale)
        nc.scalar.mul(ss[:, 248:HP], ph[:, 248:HP], -a * scale)
        self.rel(s)
        y = self.big()
        nas = -a * scale
        if K <= 248:
            nc.vector.scalar_tensor_tensor(
                out=y[:, 248 : 256 + K], in0=ph[:, 248 - K : 256],
                scalar=cK * nas, in1=ss[:, 248 : 256 + K],
                op0=Alu.mult, op1=Alu.subtract,
            )
            nc.vector.scalar_tensor_tensor(
                out=y[:, 256 + K : W], in0=ss[:, 256 : W - K], scalar=cK,
                in1=ss[:, 256 + K : W], op0=Alu.mult, op1=Alu.subtract,
            )
        else:
            pa = self.pshalo.tile([P, K - 248], F32, tag="psh")
            self.mm(pa[:, 0 : K - 248], self.Sh1[:, :], ss[:, 248 + CH - K : CH])
            nc.vector.scalar_tensor_tensor(
                out=y[:, 248:K], in0=pa[:, 0 : K - 248], scalar=cK,
                in1=ss[:, 248:K], op0=Alu.mult, op1=Alu.subtract,
            )
            nc.vector.scalar_tensor_tensor(
                out=y[:, K : 256 + K], in0=ph[:, 0:256], scalar=cK * nas,
                in1=ss[:, K : 256 + K], op0=Alu.mult, op1=Alu.subtract,
            )
            nc.vector.scalar_tensor_tensor(
                out=y[:, 256 + K : W], in0=ss[:, 256 : W - K], scalar=cK,
                in1=ss[:, 256 + K : W], op0=Alu.mult, op1=Alu.subtract,
            )
        self.rel(ss)
        return y

    def ema_pair(self, x1, i1, x2, i2, scale1=1.0, scale2=1.0):
        f1 = self.ema_front(x1, i1)
        f2 = self.ema_front(x2, i2)
        return self.ema_finish(f1, scale1), self.ema_finish(f2, scale2)

    # engine-steerable elementwise helpers (pool offload measured slower:
    # Q7 TT ops + shared SBUF port made the kernel 1.7x worse; keep on DVE)
    def ptt(self, out, in0, in1, op):
        self.nc.vector.tensor_tensor(out=out, in0=in0, in1=in1, op=op)

    def pts(self, out, in0, s1, s2, op0, op1):
        self.nc.vector.tensor_scalar(
            out=out, in0=in0, scalar1=s1, scalar2=s2, op0=op0, op1=op1
        )

    def winchain(self, xt, jmax, op):
        """doubling chain for sliding max/min: returns dict 2^j -> (tile, lo)
        where tile holds window-2^j results valid from col lo.  2^0 -> xt."""
        nc = self.nc
        chain = {1: (xt, 0)}
        cur, curlo = xt, 0
        for j in range(jmax):
            sh = 1 << j
            dst = self.big()
            nc.vector.tensor_tensor(
                out=dst[:, curlo + sh : W], in0=cur[:, curlo + sh : W],
                in1=cur[:, curlo : W - sh], op=op,
            )
            cur, curlo = dst, curlo + sh
            chain[sh * 2] = (dst, curlo)
        return chain

    def wincombine(self, chain, n, op):
        """window-n result from a doubling chain: max(M_J[t], M_J[t-r])."""
        nc = self.nc
        J = 1 << int(math.floor(math.log2(n)))
        r = n - J
        cur, curlo = chain[J]
        out = self.big()
        if r > 0:
            nc.vector.tensor_tensor(
                out=out[:, curlo + r : W], in0=cur[:, curlo + r : W],
                in1=cur[:, curlo : W - r], op=op,
            )
        else:
            nc.vector.tensor_copy(out[:, curlo:W], cur[:, curlo:W])
        return out

    def winsum18(self, xt, xlo):
        """rolling 18-window sum of xt (valid from xlo); returns tile valid
        [xlo+31, W)."""
        nc = self.nc
        tiles = []
        cur = xt
        curlo = xlo
        chain1 = None
        for j in range(4):
            sh = 1 << j
            dst = self.big()
            tiles.append(dst)
            nc.vector.tensor_add(
                dst[:, curlo + sh : W], cur[:, curlo + sh : W], cur[:, curlo : W - sh]
            )
            cur = dst
            curlo += sh
            if j == 0:
                chain1 = dst  # window-2 sums
        out = self.big()
        # S18[t] = W16[t] + W2[t-16]
        lo = curlo + 16
        nc.vector.tensor_add(out[:, lo:W], cur[:, lo:W], chain1[:, lo - 16 : W - 16])
        for t in tiles:
            self.rel(t)
        return out, lo

    # ---- full pipeline ----
    def build(self):
        nc = self.nc
        with ctile.TileContext(nc) as tc:
            with tc.tile_pool(name="big", bufs=1) as pool, tc.tile_pool(
                name="small", bufs=1
            ) as spool, tc.tile_pool(name="psc", bufs=2, space="PSUM") as pscol, tc.tile_pool(name="psh", bufs=4, space="PSUM") as pshalo, \
                 tc.tile_pool(name="psr", bufs=2, space="PSUM") as psrow:
                self.pool = pool
                self.spool = spool
                self.pscol = pscol
                self.psrow = psrow
                self.pshalo = pshalo
                self.emit()
        nc.finalize()
        return nc

    def mm(self, out, lhsT, rhs):
        self.nc.tensor.matmul(out, lhsT, rhs, start=True, stop=True)

    def emit(self):
        nc = self.nc
        I32 = mybir.dt.int32
        self.eps8 = self.spool.tile([P, 1], F32, tag="c_eps8")
        nc.gpsimd.memset(self.eps8[:, :], 1e-8)
        self.ONESC = self.spool.tile([P, 1], F32, tag="c_ones_col")
        nc.gpsimd.memset(self.ONESC[:, :], 1.0)
        self.nanch = self.spool.tile([P, 1], F32, tag="c_nanch")
        nc.gpsimd.memset(self.nanch[:, :], -self.anchor)
        # PE helper constants: ii[p, m] = m - p; Sh1 = (ii==1); Ident = (ii==0)
        ii = self.spool.tile([P, P], I32, tag="c_iota")
        nc.gpsimd.iota(ii[:, :], pattern=[[1, P]], base=0, channel_multiplier=-1)
        self.Sh1 = self.spool.tile([P, P], F32, tag="c_sh1")
        nc.vector.tensor_single_scalar(
            out=self.Sh1[:, :], in_=ii[:, :], scalar=1, op=Alu.is_equal
        )
        self.Ident = self.spool.tile([P, P], F32, tag="c_ident")
        nc.vector.tensor_single_scalar(
            out=self.Ident[:, :], in_=ii[:, :], scalar=0, op=Alu.is_equal
        )
        # second-order carry matrices for slow alphas (cF > 1e-10)
        self.M2 = {}
        sh2 = None
        for i, a in enumerate(self.alphas):
            cF = (1.0 - a) ** CH
            if cF > 1e-10:
                if sh2 is None:
                    sh2 = self.spool.tile([P, P], F32, tag="c_sh2")
                    nc.vector.tensor_single_scalar(
                        out=sh2[:, :], in_=ii[:, :], scalar=2, op=Alu.is_equal
                    )
                m = self.spool.tile([P, P], F32, tag=f"c_m2_{i}")
                nc.vector.scalar_tensor_tensor(
                    out=m[:, :], in0=sh2[:, :], scalar=float(cF), in1=self.Sh1[:, :],
                    op0=Alu.mult, op1=Alu.add,
                )
                self.M2[i] = m
        # per-conv scan multiplier constants: CONSTS[:, i] = 1 - alpha_i
        self.CONSTS = self.spool.tile([P, len(self.alphas)], F32, tag="c_scanmul")
        for i, a in enumerate(self.alphas):
            nc.gpsimd.memset(self.CONSTS[:, i : i + 1], 1.0 - a)
        self.ones11 = self.spool.tile([1, 1], F32, tag="c_ones11")
        nc.gpsimd.memset(self.ones11[:, :], 1.0)
        self.m50 = self.spool.tile([P, 1], F32, tag="c_m50")
        nc.gpsimd.memset(self.m50[:, :], -50.0)
        self.TG = self.spool.tile([P, CH], I32, tag="c_tg")
        nc.gpsimd.iota(self.TG[:, :], pattern=[[1, CH]], base=0, channel_multiplier=CH)
        Ht = self.load_series(self.DH)
        Lt = self.load_series(self.DL, eng="gpsimd")
        Ct = self.load_series(self.DC)

        # --- KDJ blocks ---
        hchain = self.winchain(Ht, 7, Alu.max)
        lchain = self.winchain(Lt, 7, Alu.min)

        def rsvf(nw):
            hh = self.wincombine(hchain, nw, Alu.max)
            ll = self.wincombine(lchain, nw, Alu.min)
            hl = self.big()
            nc.vector.tensor_sub(hl[:, C0:W], hh[:, C0:W], ll[:, C0:W])
            nc.vector.tensor_scalar_max(hl[:, C0:W], hl[:, C0:W], 1e-8)
            rcp = self.big()
            nc.vector.reciprocal_approx_fast(out=rcp[:, C0:W], in_=hl[:, C0:W])
            num = self.big()
            nc.vector.tensor_sub(num[:, C0:W], Ct[:, C0:W], ll[:, C0:W])
            r0 = self.big()
            nc.vector.tensor_mul(r0[:, C0:W], num[:, C0:W], rcp[:, C0:W])
            self.rel(hh, ll, hl, rcp, num)
            rsv01 = self.big()
            nc.vector.tensor_scalar(
                out=rsv01[:, C0:W], in0=r0[:, C0:W], scalar1=0.0, scalar2=1.0,
                op0=Alu.max, op1=Alu.min,
            )
            self.rel(r0)
            return rsv01

        def jof(Kv, Dv, rows):
            Jv = self.big()
            dkd = self.big()
            self.ptt(dkd[:, 248:W], Kv[:, 248:W], Dv[:, 248:W], Alu.subtract)
            nc.vector.scalar_tensor_tensor(
                out=Jv[:, 248:W], in0=dkd[:, 248:W], scalar=2.0, in1=Kv[:, 248:W],
                op0=Alu.mult, op1=Alu.add,
            )
            self.rel(dkd)
            for ridx, tt_ in zip(rows, (Kv, Dv, Jv)):
                if ridx is not None:
                    self.store_row(ridx, tt_)
            return Jv

        rsv1 = rsvf(204)
        rsv2 = rsvf(18)
        K1, K2 = self.ema_pair(rsv1, 6, rsv2, 8, 100.0, 100.0)
        self.rel(rsv1, rsv2)
        D1, D2 = self.ema_pair(K1, 7, K2, 9)
        J1 = jof(K1, D1, (9, 10, 11))
        J2 = jof(K2, D2, (12, 13, 14))
        self.rel(K1, D1, K2, D2)
        # --- TEMA3 + TEMAP2 chains (stage-interleaved ema pairs) ---
        EMA1, E21 = self.ema_pair(Ct, 0, Ct, 3)
        EMA2, E221 = self.ema_pair(EMA1, 1, E21, 4)
        EMA3, E231 = self.ema_pair(EMA2, 2, E221, 5)
        TEMA3 = self.big()
        d = self.big()
        self.ptt(d[:, 248:W], EMA1[:, 248:W], EMA2[:, 248:W], Alu.subtract)
        nc.vector.scalar_tensor_tensor(
            out=TEMA3[:, 248:W], in0=d[:, 248:W], scalar=3.0, in1=EMA3[:, 248:W],
            op0=Alu.mult, op1=Alu.add,
        )
        self.rel(EMA1, EMA2, EMA3, d)
        self.store_row(4, TEMA3)
        TEMAP2 = self.big()
        d = self.big()
        self.ptt(d[:, 248:W], E21[:, 248:W], E221[:, 248:W], Alu.subtract)
        nc.vector.scalar_tensor_tensor(
            out=TEMAP2[:, 248:W], in0=d[:, 248:W], scalar=3.0, in1=E231[:, 248:W],
            op0=Alu.mult, op1=Alu.add,
        )
        self.rel(E21, E221, E231, d)

        # --- stdp(C,18) via per-partition-anchored rolling cumsums ---
        # anchor per partition (C at col 232) kills catastrophic cancellation
        npc = self.small()
        nc.scalar.mul(npc[:, 0:1], Ct[:, 232:233], -1.0)
        dev = self.big()
        nc.scalar.activation(dev[:, 232:W], Ct[:, 232:W], Act.Identity, bias=npc[:, 0:1])
        dev2 = self.big()
        nc.scalar.activation(dev2[:, 232:W], Ct[:, 232:W], Act.Square, bias=npc[:, 0:1])
        NS = W - 232
        ones_bc = self.ONESC[:, 0:1].broadcast_to([P, NS])
        cs = self.big()
        nc.vector.tensor_tensor_scan(
            out=cs[:, 232:W], data0=ones_bc, data1=dev[:, 232:W],
            initial=0.0, op0=Alu.mult, op1=Alu.add,
        )
        cs2 = self.big()
        nc.vector.tensor_tensor_scan(
            out=cs2[:, 232:W], data0=ones_bc, data1=dev2[:, 232:W],
            initial=0.0, op0=Alu.mult, op1=Alu.add,
        )
        self.rel(dev, dev2)
        m = self.big()
        nc.vector.scalar_tensor_tensor(
            out=m[:, 250:W], in0=cs[:, 232 : W - 18], scalar=-1.0,
            in1=cs[:, 250:W], op0=Alu.mult, op1=Alu.add,
        )
        nc.scalar.mul(m[:, 250:W], m[:, 250:W], 1.0 / 18.0)
        ex2 = self.big()
        nc.vector.scalar_tensor_tensor(
            out=ex2[:, 250:W], in0=cs2[:, 232 : W - 18], scalar=-1.0,
            in1=cs2[:, 250:W], op0=Alu.mult, op1=Alu.add,
        )
        nc.scalar.mul(ex2[:, 250:W], ex2[:, 250:W], 1.0 / 18.0)
        self.rel(cs, cs2)
        mm = self.big()
        nc.scalar.square(mm[:, 250:W], m[:, 250:W])
        var = self.big()
        self.ptt(var[:, 250:W], ex2[:, 250:W], mm[:, 250:W], Alu.subtract)
        varc = self.big()
        nc.vector.tensor_scalar_max(varc[:, 250:W], var[:, 250:W], 0.0)
        DIS = self.big()
        nc.scalar.activation(DIS[:, 250:W], varc[:, 250:W], Act.Sqrt)
        self.rel(m, ex2, mm, var, varc)
        TEU3 = self.big()
        self.ptt(TEU3[:, C0:W], TEMA3[:, C0:W], DIS[:, C0:W], Alu.add)
        TED = self.big()
        self.ptt(TED[:, C0:W], TEMA3[:, C0:W], DIS[:, C0:W], Alu.subtract)
        self.store_row(3, TEU3)
        self.store_row(5, TED)
        self.rel(DIS, TEU3, TED)

        rsv3 = rsvf(9)
        rsvn = rsvf(36)
        K3, KN3 = self.ema_pair(rsv3, 10, rsvn, 12, 100.0, 100.0)
        self.rel(rsv3, rsvn)
        D3, DN3x = self.ema_pair(K3, 11, KN3, 13, 1.0, 2.0)
        J3 = jof(K3, D3, (15, 16, 17))
        JN3 = self.big()
        nc.vector.scalar_tensor_tensor(
            out=JN3[:, 248:W], in0=KN3[:, 248:W], scalar=3.0,
            in1=DN3x[:, 248:W], op0=Alu.mult, op1=Alu.subtract,
        )
        self.store_row(18, JN3)
        self.rel(K3, D3, KN3, DN3x, JN3)
        for ch_ in (hchain, lchain):
            for kk, (tt_, _) in ch_.items():
                if kk > 1:
                    self.rel(tt_)
        self.rel(Ht, Lt)



        # --- T ratios ---
        def tdiff(xt, lag, row_idx, lo=254):
            dt_ = self.big()
            nc.vector.tensor_tensor(
                out=dt_[:, lo:W], in0=xt[:, lo:W], in1=xt[:, lo - lag : W - lag],
                op=Alu.subtract,
            )
            ab = self.big()
            nc.scalar.activation(ab[:, lo:W], xt[:, lo - lag : W - lag], Act.Abs)
            abe = self.big()
            nc.scalar.activation(abe[:, lo:W], ab[:, lo:W], Act.Identity, bias=self.eps8[:, 0:1])
            rr = self.big()
            nc.vector.reciprocal_approx_fast(out=rr[:, lo:W], in_=abe[:, lo:W])
            ts_ = self.big()
            nc.vector.tensor_mul(ts_[:, lo:W], dt_[:, lo:W], rr[:, lo:W])
            self.rel(dt_, ab, abe, rr)
            if row_idx is not None:
                self.store_row(row_idx, ts_)
            return ts_

        T3s = tdiff(TEMA3, 6, 8)
        T1s = tdiff(TEMA3, 1, 6)
        T2s = tdiff(TEMAP2, 6, 7)
        self.rel(TEMAP2, TEMA3)

        # --- JX family ---
        JXb = self.big()
        u = self.big()
        nc.vector.tensor_mul(u[:, 254:W], J3[:, 254:W], T1s[:, 254:W])
        v = self.big()
        self.ptt(v[:, 254:W], J1[:, 254:W], J2[:, 254:W], Alu.add)
        self.ptt(JXb[:, 254:W], u[:, 254:W], v[:, 254:W], Alu.add)
        self.rel(u, v, J3, T1s)
        F1 = self.big()
        nc.vector.tensor_mul(F1[:, 254:W], J2[:, 254:W], T3s[:, 254:W])
        self.rel(J2, T3s)
        F2 = self.big()
        self.ptt(F2[:, 254:W], J1[:, 254:W], T2s[:, 254:W], Alu.mult)
        self.rel(J1, T2s)
        self.store_row(19, JXb)
        self.store_row(20, F1)
        self.store_row(21, F2)

        EMA_JX, EMA_F1 = self.ema_pair(JXb, 14, F1, 15)
        EMA_F2, EMA8_JX = self.ema_pair(F2, 16, JXb, 17)
        EMA8_F1, EMA8_F2 = self.ema_pair(F1, 18, F2, 19)
        self.store_row(22, EMA_JX)
        self.store_row(23, EMA_F1)
        self.store_row(24, EMA_F2)

        def jx_combine(base, f1, f2, row_idx, lo=254):
            w_ = self.big()
            self.ptt(w_[:, lo:W], f1[:, lo:W], f2[:, lo:W], Alu.add)
            z = self.big()
            nc.vector.scalar_tensor_tensor(
                out=z[:, lo:W], in0=w_[:, lo:W], scalar=6.0, in1=base[:, lo:W],
                op0=Alu.mult, op1=Alu.add,
            )
            out = self.big()
            nc.scalar.activation(out[:, lo:W], z[:, lo:W], Act.Identity, bias=self.m50[:, 0:1])
            self.rel(w_, z)
            self.store_row(row_idx, out)
            return out

        # need col 255 for the cross lag -> compute from col 254
        JX = jx_combine(JXb, F1, F2, 27, lo=254)
        EMAJX = jx_combine(EMA_JX, EMA_F1, EMA_F2, 28, lo=254)
        EMAJX8 = jx_combine(EMA8_JX, EMA8_F1, EMA8_F2, 29, lo=254)
        self.rel(JXb, F1, F2, EMA_JX, EMA_F1, EMA_F2, EMA8_JX, EMA8_F1, EMA8_F2)
        self.rel(EMAJX8)

        # --- crosses + segmented MA scans ---
        def macond(updown):
            g = self.big()
            l = self.big()
            if updown == "up":
                nc.vector.tensor_tensor(
                    out=g[:, 255:W], in0=JX[:, 255:W], in1=EMAJX[:, 255:W],
                    op=Alu.is_gt,
                )
                nc.vector.tensor_tensor(
                    out=l[:, 255:W], in0=JX[:, 254 : W - 1],
                    in1=EMAJX[:, 254 : W - 1], op=Alu.is_le,
                )
            else:
                nc.vector.tensor_tensor(
                    out=g[:, 255:W], in0=JX[:, 255:W], in1=EMAJX[:, 255:W],
                    op=Alu.is_lt,
                )
                nc.vector.tensor_tensor(
                    out=l[:, 255:W], in0=JX[:, 254 : W - 1],
                    in1=EMAJX[:, 254 : W - 1], op=Alu.is_ge,
                )
            cond = self.big()
            self.ptt(cond[:, 255:W], g[:, 255:W], l[:, 255:W], Alu.mult)
            m_ = self.big()
            self.pts(m_[:, 255:W], cond[:, 255:W], -1.0, 1.0, Alu.mult, Alu.add)
            self.rel(g, l)
            return cond, m_

        # Manual expansion (cnt first to derive A, then S and seen share it)
        for updown, row_idx, diag_idx in (("dn", 25, 1), ("up", 26, 0)):
            cond, m_ = macond(updown)
            dmask = self.big()
            self.ptt(dmask[:, C0:W], Ct[:, C0:W], m_[:, C0:W], Alu.mult)

            # --- cnt: scan1, A row via PE transpose, affine chain, scan2 ---
            cnt_s = self.big()
            nc.vector.tensor_tensor_scan(
                out=cnt_s[:, C0:W], data0=m_[:, C0:W], data1=m_[:, C0:W],
                initial=0.0, op0=Alu.mult, op1=Alu.add,
            )
            acol = self.small()
            nc.vector.tensor_single_scalar(
                out=acol[:, 0:1], in_=cnt_s[:, W - 1 : W], scalar=float(CH),
                op=Alu.is_ge,
            )
            par = self.psrow.tile([1, P], F32, tag="psrow")
            self.mm(par[0:1, 0:P], acol[:, 0:1], self.Ident[:, :])
            arow = self.row()
            nc.vector.tensor_copy(arow[0:1, 0:P], par[0:1, 0:P])
            self.rels(acol)

            def chain(scan1_tile, op0, op1, d0, d1):
                """affine partition chain for a scan1 result, then scan2
                in place (initial from PSUM via PE shift-back)."""
                rowt = self.row129()
                nc.gpsimd.memset(rowt[0:1, 0:1], 0.0)
                pbr = self.psrow.tile([1, P], F32, tag="psrow")
                self.mm(pbr[0:1, 0:P], scan1_tile[:, W - 1 : W], self.Ident[:, :])
                nc.vector.tensor_tensor_scan(
                    out=rowt[0:1, 1 : P + 1], data0=arow[0:1, 0:P],
                    data1=pbr[0:1, 0:P], initial=0.0, op0=Alu.mult, op1=Alu.add,
                )
                pcc = self.pscol.tile([P, 1], F32, tag="pscol")
                self.mm(pcc[:, 0:1], rowt[0:1, 0:P], self.ones11[0:1, 0:1])
                nc.vector.tensor_tensor_scan(
                    out=scan1_tile[:, C0:W], data0=d0, data1=d1,
                    initial=pcc[:, 0:1], op0=op0, op1=op1,
                )
                self.relr129(rowt)

            chain(cnt_s, Alu.mult, Alu.add, m_[:, C0:W], m_[:, C0:W])

            # --- S: sum since last event ---
            Ssum = self.big()
            nc.vector.tensor_tensor_scan(
                out=Ssum[:, C0:W], data0=m_[:, C0:W], data1=dmask[:, C0:W],
                initial=0.0, op0=Alu.mult, op1=Alu.add,
            )
            chain(Ssum, Alu.mult, Alu.add, m_[:, C0:W], dmask[:, C0:W])

            # --- seen: cnt counts from ext start when no event has occurred,
            # so seen[t] = (cnt[t] <= ext_index[t]) ---
            seen = self.big()
            dcol = self.small()
            nc.vector.scalar_tensor_tensor(
                out=seen[:, C0:W], in0=cnt_s[:, C0:W], scalar=1.0,
                in1=self.TG[:, 0:CH], op0=Alu.mult, op1=Alu.is_le,
                accum_out=dcol[:, 0:1],
            )
            self.relr(arow)
            self.rel(cond, m_, dmask)

            # ma = (S * recip(max(cnt,1))) * seen
            rc = self.big()
            nc.vector.tensor_scalar_max(rc[:, C0:W], cnt_s[:, C0:W], 1.0)
            rcp = self.big()
            nc.vector.reciprocal_approx_fast(out=rcp[:, C0:W], in_=rc[:, C0:W])
            ma0 = self.big()
            self.ptt(ma0[:, C0:W], Ssum[:, C0:W], rcp[:, C0:W], Alu.mult)
            ma = self.big()
            nc.vector.tensor_mul(ma[:, C0:W], ma0[:, C0:W], seen[:, C0:W])
            self.rel(rc, rcp, ma0, cnt_s, Ssum)
            self.store_row(row_idx, ma)

            # diag: min of seen over partitions 5..127 (covers valid region)
            drow = self.row()
            nc.sync.dma_start(out=drow[0:1, 0 : P - 1], in_=dcol[1:P, 0:1])
            done = self.spool.tile([1, 1], F32, tag=f"diag{diag_idx}")
            nc.vector.tensor_reduce(
                out=done[0:1, 0:1], in_=drow[0:1, 0 : P - 1],
                axis=mybir.AxisListType.X, op=Alu.min,
            )
            self.relr(drow)
            nc.sync.dma_start(
                out=self.DIAG[diag_idx : diag_idx + 1].rearrange(
                    "(a b) -> a b", a=1, b=1
                ),
                in_=done[0:1, 0:1],
            )
            self.rels(dcol)
            self.rel(seen, ma)

        self.rel(Ct, JX, EMAJX)


_CACHE = {}


def _build(alphas, anchor):
    key = (tuple(round(float(a), 12) for a in alphas), round(float(anchor), 6))
    if key not in _CACHE:
        kb = KB(alphas, anchor)
        _CACHE[key] = kb.build()
    return _CACHE[key]


def _shard(x):
    """per-core input arrays [DLEN], clamp-padded on the global left."""
    outs = []
    for mcore in range(NCORES):
        lo = (mcore + 1) * S - DLEN
        if lo < 0:
            d = np.concatenate(
                [np.full(-lo, x[0], np.float32), x[0 : (mcore + 1) * S]]
            )
        else:
            d = x[lo : (mcore + 1) * S]
        outs.append(np.ascontiguousarray(d, np.float32))
    return outs


def _host_ma(C, JX, EJ):
    """exact host fallback for ma rows (numpy, global)."""
    f32 = np.float32
    T_ = len(C)
    lag = lambda x: np.concatenate([x[:1], x[:-1]])
    JXp, EJp = lag(JX), lag(EJ)
    res = {}
    cs = np.concatenate([[0.0], np.cumsum(C.astype(np.float64))])
    t_idx = np.arange(T_)
    for key, cond in (
        ("dn", (JX < EJ) & (JXp >= EJp)),
        ("up", (JX > EJ) & (JXp <= EJp)),
    ):
        last = np.maximum.accumulate(np.where(cond, t_idx, -1))
        csl = cs[np.maximum(last, 0) + 1]
        s = cs[t_idx + 1] - csl
        n = t_idx - last
        res[key] = np.where(
            (last >= 0) & (n > 0), s / np.maximum(n, 1), 0.0
        ).astype(f32)
    return res["dn"], res["up"]


def run_cores(inputs, trace=False):
    """compile (cached) + run on 8 cores; returns (results, BassKernelResults)."""
    C = np.ascontiguousarray(inputs["C"], np.float32)
    H = np.ascontiguousarray(inputs["H"], np.float32)
    L = np.ascontiguousarray(inputs["L"], np.float32)
    w = np.asarray(inputs["w_alphas"], np.float32)
    alphas = [float(1.0 / (1.0 + math.exp(-float(x)))) for x in w]
    nc = _build(alphas, float(C[0]))
    dc, dh, dl = _shard(C), _shard(H), _shard(L)
    in_maps = [
        {"DC": dc[m], "DH": dh[m], "DL": dl[m]} for m in range(NCORES)
    ]
    res = run_bass_kernel_spmd(
        nc, in_maps, core_ids=list(range(NCORES)), trace=trace
    )
    return res


def kernel(C, H, L, w_alphas):
    inputs = {"C": C, "H": H, "L": L, "w_alphas": w_alphas}
    res = run_cores(inputs)
    outs = [res.results[m]["OUT"].reshape(NROWS, EXT)[:, HALO:] for m in range(NCORES)]
    full = np.concatenate(outs, axis=1)
    full[0] = np.asarray(C, np.float32)
    full[1] = np.asarray(H, np.float32)
    full[2] = np.asarray(L, np.float32)

    # host patch: reference's partial-window std for the first 17 bars
    Cg = np.asarray(C, np.float64)[:17]
    for t in range(17):
        wdw = Cg[: t + 1]
        dis = math.sqrt(max(np.mean(wdw * wdw) - np.mean(wdw) ** 2, 0.0))
        full[3, t] = np.float32(full[4, t] + dis)
        full[5, t] = np.float32(full[4, t] - dis)

    # diag check: cross gap exceeded the halo on some core -> exact host fix
    need_fix = False
    for mcore in range(1, NCORES):
        dg = res.results[mcore]["DIAG"]
        if dg.min() < CH - 0.5:
            need_fix = True
    if need_fix:
        ma_dn, ma_up = _host_ma(
            np.asarray(C, np.float32), full[27], full[28]
        )
        full[25] = ma_dn
        full[26] = ma_up
    return full.astype(np.float32)



# revision 19
# speedup vs baseline: 1.1941x; 1.1941x over previous
"""Trainium2 Bass kernel for nn_DifferentiableFeatureExtractor.

Strategy (8 NeuronCores, shard T=1048576 along time):
  - per-core extended domain EXT = S + 2048 halo = 133120 = 128 partitions x 1040
  - each partition holds a contiguous 1072-bar chunk plus a 256-bar AP halo
    (tile [128, 1328]); host supplies a 256-bar lead-in so partition 0's halo
    is real data (clamp-padded at the global left edge like the reference)
  - 20 truncated-EMA convs as fp32 recurrences: y = a*(s - c^K s[t-K]) with
    s from tensor_tensor_scan (2-pass blocked scan, partition carry chained
    via shifted end-columns; c^CH second-order term kept when significant)
  - sliding max/min via log-doubling with shifted APs (halo-local)
  - rolling std via anchored windowed sums (doubling)
  - BARSLAST/MA_DYNAMIC as segmented scans (reset at cross events) with
    affine partition-carry chains; exact whenever the previous cross lies
    within the 2048-bar halo (diag output flags violations -> host fallback)
All heavy compute runs on device; host only shards, gathers, patches the
17 reference partial-window std bars, and handles the (never-observed)
diag fallback.
"""
import math

import numpy as np

import concourse.bacc as bacc
from concourse.bass_types import AP as BassAP
import concourse.mybir as mybir
from concourse import tile as ctile
from concourse.bass_utils import run_bass_kernel_spmd

F32 = mybir.dt.float32
Alu = mybir.AluOpType
Act = mybir.ActivationFunctionType

T = 1048576
NCORES = 8
S = T // NCORES            # 131072
P = 128
CH = 1040                  # chunk cols per partition
HP = 256                   # per-partition halo cols
W = HP + CH                # 1328
EXT = P * CH               # 133120
HALO = EXT - S             # 2048
DLEN = HP + EXT            # 133376
C0 = HP                    # chunk start col
NROWS = 30

# static truncation lengths (depend only on the reference's constant ALPHAS)
KS = [72, 72, 72, 286, 286, 286, 559, 89, 54, 47, 40, 30, 130, 30,
      30, 30, 30, 37, 37, 37]


class KB:
    """kernel builder with a simple big-tile freelist"""

    def __init__(self, alphas, anchor):
        self.alphas = [float(a) for a in alphas]
        self.anchor = float(anchor)
        nc = bacc.Bacc(None, target_bir_lowering=False)
        self.nc = nc
        self.DC = nc.dram_tensor("DC", [DLEN], F32, kind="ExternalInput")
        self.DH = nc.dram_tensor("DH", [DLEN], F32, kind="ExternalInput")
        self.DL = nc.dram_tensor("DL", [DLEN], F32, kind="ExternalInput")
        self.OUT = nc.dram_tensor("OUT", [NROWS * EXT], F32, kind="ExternalOutput")
        self.DIAG = nc.dram_tensor("DIAG", [2], F32, kind="ExternalOutput")
        self.free_big = []
        self.n_big = 0
        self.free_small = []
        self.n_small = 0
        self.free_row = []
        self.n_row = 0
        self.free_row129 = []
        self.n_row129 = 0

    # ---- tile management ----
    def big(self):
        if self.free_big:
            return self.free_big.pop(0)
        t = self.pool.tile([P, W], F32, tag=f"big{self.n_big}")
        self.n_big += 1
        return t

    def rel(self, *ts):
        for t in ts:
            self.free_big.append(t)

    def small(self):
        if self.free_small:
            return self.free_small.pop()
        t = self.spool.tile([P, 1], F32, tag=f"small{self.n_small}")
        self.n_small += 1
        return t

    def rels(self, *ts):
        for t in ts:
            self.free_small.append(t)

    def row(self):
        if self.free_row:
            return self.free_row.pop()
        t = self.spool.tile([1, P], F32, tag=f"row{self.n_row}")
        self.n_row += 1
        return t

    def relr(self, *ts):
        for t in ts:
            self.free_row.append(t)

    def row129(self):
        if self.free_row129:
            return self.free_row129.pop()
        t = self.spool.tile([1, P + 1], F32, tag=f"row129_{self.n_row129}")
        self.n_row129 += 1
        return t

    def relr129(self, *ts):
        for t in ts:
            self.free_row129.append(t)

    # ---- IO ----
    def load_series(self, dram, eng="sync"):
        nc = self.nc
        t = self.big()
        base = dram[0:DLEN].rearrange("(a b) -> a b", a=1, b=DLEN)
        src_ap = BassAP(base.tensor, 0, [[CH, P], [1, W]])
        getattr(nc, eng).dma_start(out=t[:, 0:W], in_=src_ap)
        return t

    def store_row(self, r, t):
        nc = self.nc
        nc.sync.dma_start(
            out=self.OUT[r * EXT : (r + 1) * EXT].rearrange(
                "(p w) -> p w", p=P, w=CH
            ),
            in_=t[:, C0:W],
        )

    # ---- building blocks ----
    def ema(self, xt, i, scale=1.0):
        """truncated EMA of xt (valid on chunk cols) -> new tile valid [248, W).
        Output scaled by `scale`."""
        nc = self.nc
        a = self.alphas[i]
        K = KS[i]
        c = 1.0 - a
        cF = float(c) ** CH
        cK = float(c) ** K

        s = self.big()
        # scan1: local scan over a truncation-sufficient tail window
        V = min(CH, 2 * K + 16) if i < 6 else min(CH, 2 * K + 16, K + 64)
        cbc = self.CONSTS[:, i : i + 1].broadcast_to([P, V])
        nc.vector.tensor_tensor_scan(
            out=s[:, W - V : W], data0=cbc,
            data1=xt[:, W - V : W], initial=0.0, op0=Alu.mult, op1=Alu.add,
        )
        # partition carry via PE shift-matmul: carry[p] = E[p-1] (+cF*E[p-2])
        mmat = self.M2.get(i, self.Sh1)
        pcar = self.pscol.tile([P, 1], F32, tag="pscol")
        self.mm(pcar[:, 0:1], mmat[:, :], s[:, W - 1 : W])
        # scan2: chained scan with PSUM initial
        cbc2 = self.CONSTS[:, i : i + 1].broadcast_to([P, CH])
        nc.vector.tensor_tensor_scan(
            out=s[:, C0:W], data0=cbc2,
            data1=xt[:, C0:W], initial=pcar[:, 0:1], op0=Alu.mult, op1=Alu.add,
        )
        # halo values via PE one-partition shift of the chunk tail; the
        # corrections read them straight from PSUM (no halo-ACT pass)
        ph = self.pshalo.tile([P, HP], F32, tag="psh")
        self.mm(ph[:, 0:HP], self.Sh1[:, :], s[:, CH:W])
        ss = self.big()
        # ss chunk region + the 8 halo cols [248, HP) the corrections read
        nc.scalar.mul(ss[:, HP:W], s[:, C0:W], -a * scale)
        nc.scalar.mul(ss[:, 248:HP], ph[:, 248:HP], -a * scale)
        self.rel(s)
        y = self.big()
        nas = -a * scale
        if K <= 248:
            # head (cols [248, 256+K)): ss[t-K] lives in the halo -> PSUM ph
            nc.vector.scalar_tensor_tensor(
                out=y[:, 248 : 256 + K], in0=ph[:, 248 - K : 256],
                scalar=cK * nas, in1=ss[:, 248 : 256 + K],
                op0=Alu.mult, op1=Alu.subtract,
            )
            # main (cols [256+K, W)): chunk-only reads
            nc.vector.scalar_tensor_tensor(
                out=y[:, 256 + K : W], in0=ss[:, 256 : W - K], scalar=cK,
                in1=ss[:, 256 + K : W], op0=Alu.mult, op1=Alu.subtract,
            )
        else:
            # piece A (cols [248, K)): one-partition shift via PE (unscaled s
            # tail was consumed before rel; shift ss chunk instead)
            pa = self.pshalo.tile([P, K - 248], F32, tag="psh")
            self.mm(pa[:, 0 : K - 248], self.Sh1[:, :], ss[:, 248 + CH - K : CH])
            nc.vector.scalar_tensor_tensor(
                out=y[:, 248:K], in0=pa[:, 0 : K - 248], scalar=cK,
                in1=ss[:, 248:K], op0=Alu.mult, op1=Alu.subtract,
            )
            # piece B1 (cols [K, 256+K)): halo reads from PSUM ph
            nc.vector.scalar_tensor_tensor(
                out=y[:, K : 256 + K], in0=ph[:, 0:256], scalar=cK * nas,
                in1=ss[:, K : 256 + K], op0=Alu.mult, op1=Alu.subtract,
            )
            # piece B2 (cols [256+K, W)): same-partition chunk AP shift
            nc.vector.scalar_tensor_tensor(
                out=y[:, 256 + K : W], in0=ss[:, 256 : W - K], scalar=cK,
                in1=ss[:, 256 + K : W], op0=Alu.mult, op1=Alu.subtract,
            )
        self.rel(ss)
        return y

    def ema_front(self, xt, i):
        """scan stages of ema(); returns state for ema_finish."""
        nc = self.nc
        a = self.alphas[i]
        K = KS[i]
        s = self.big()
        V = min(CH, 2 * K + 16) if i < 6 else min(CH, 2 * K + 16, K + 64)
        cbc = self.CONSTS[:, i : i + 1].broadcast_to([P, V])
        nc.vector.tensor_tensor_scan(
            out=s[:, W - V : W], data0=cbc,
            data1=xt[:, W - V : W], initial=0.0, op0=Alu.mult, op1=Alu.add,
        )
        mmat = self.M2.get(i, self.Sh1)
        pcar = self.pscol.tile([P, 1], F32, tag="pscol")
        self.mm(pcar[:, 0:1], mmat[:, :], s[:, W - 1 : W])
        cbc2 = self.CONSTS[:, i : i + 1].broadcast_to([P, CH])
        nc.vector.tensor_tensor_scan(
            out=s[:, C0:W], data0=cbc2,
            data1=xt[:, C0:W], initial=pcar[:, 0:1], op0=Alu.mult, op1=Alu.add,
        )
        return (s, i)

    def ema_finish(self, state, scale=1.0):
        """ACT scale + correction stages of ema()."""
        nc = self.nc
        s, i = state
        a = self.alphas[i]
        K = KS[i]
        c = 1.0 - a
        cK = float(c) ** K
        ph = self.pshalo.tile([P, HP], F32, tag="psh")
        self.mm(ph[:, 0:HP], self.Sh1[:, :], s[:, CH:W])
        ss = self.big()
        nc.scalar.mul(ss[:, HP:W], s[:, C0:W], -a * scale)
        nc.scalar.mul(ss[:, 248:HP], ph[:, 248:HP], -a * scale)
        self.rel(s)
        y = self.big()
        nas = -a * scale
        if K <= 248:
            nc.vector.scalar_tensor_tensor(
                out=y[:, 248 : 256 + K], in0=ph[:, 248 - K : 256],
                scalar=cK * nas, in1=ss[:, 248 : 256 + K],
                op0=Alu.mult, op1=Alu.subtract,
            )
            nc.vector.scalar_tensor_tensor(
                out=y[:, 256 + K : W], in0=ss[:, 256 : W - K], scalar=cK,
                in1=ss[:, 256 + K : W], op0=Alu.mult, op1=Alu.subtract,
            )
        else:
            pa = self.pshalo.tile([P, K - 248], F32, tag="psh")
            self.mm(pa[:, 0 : K - 248], self.Sh1[:, :], ss[:, 248 + CH - K : CH])
            nc.vector.scalar_tensor_tensor(
                out=y[:, 248:K], in0=pa[:, 0 : K - 248], scalar=cK,
                in1=ss[:, 248:K], op0=Alu.mult, op1=Alu.subtract,
            )
            nc.vector.scalar_tensor_tensor(
                out=y[:, K : 256 + K], in0=ph[:, 0:256], scalar=cK * nas,
                in1=ss[:, K : 256 + K], op0=Alu.mult, op1=Alu.subtract,
            )
            nc.vector.scalar_tensor_tensor(
                out=y[:, 256 + K : W], in0=ss[:, 256 : W - K], scalar=cK,
                in1=ss[:, 256 + K : W], op0=Alu.mult, op1=Alu.subtract,
            )
        self.rel(ss)
        return y

    def ema_pair(self, x1, i1, x2, i2, scale1=1.0, scale2=1.0):
        f1 = self.ema_front(x1, i1)
        f2 = self.ema_front(x2, i2)
        return self.ema_finish(f1, scale1), self.ema_finish(f2, scale2)

    # engine-steerable elementwise helpers (pool offload measured slower:
    # Q7 TT ops + shared SBUF port made the kernel 1.7x worse; keep on DVE)
    def ptt(self, out, in0, in1, op):
        self.nc.vector.tensor_tensor(out=out, in0=in0, in1=in1, op=op)

    def pts(self, out, in0, s1, s2, op0, op1):
        self.nc.vector.tensor_scalar(
            out=out, in0=in0, scalar1=s1, scalar2=s2, op0=op0, op1=op1
        )

    def winchain(self, xt, jmax, op):
        """doubling chain for sliding max/min: returns dict 2^j -> (tile, lo)
        where tile holds window-2^j results valid from col lo.  2^0 -> xt."""
        nc = self.nc
        chain = {1: (xt, 0)}
        cur, curlo = xt, 0
        for j in range(jmax):
            sh = 1 << j
            dst = self.big()
            nc.vector.tensor_tensor(
                out=dst[:, curlo + sh : W], in0=cur[:, curlo + sh : W],
                in1=cur[:, curlo : W - sh], op=op,
            )
            cur, curlo = dst, curlo + sh
            chain[sh * 2] = (dst, curlo)
        return chain

    def wincombine(self, chain, n, op):
        """window-n result from a doubling chain: max(M_J[t], M_J[t-r])."""
        nc = self.nc
        J = 1 << int(math.floor(math.log2(n)))
        r = n - J
        cur, curlo = chain[J]
        out = self.big()
        if r > 0:
            nc.vector.tensor_tensor(
                out=out[:, curlo + r : W], in0=cur[:, curlo + r : W],
                in1=cur[:, curlo : W - r], op=op,
            )
        else:
            nc.vector.tensor_copy(out[:, curlo:W], cur[:, curlo:W])
        return out

    def winsum18(self, xt, xlo):
        """rolling 18-window sum of xt (valid from xlo); returns tile valid
        [xlo+31, W)."""
        nc = self.nc
        tiles = []
        cur = xt
        curlo = xlo
        chain1 = None
        for j in range(4):
            sh = 1 << j
            dst = self.big()
            tiles.append(dst)
            nc.vector.tensor_add(
                dst[:, curlo + sh : W], cur[:, curlo + sh : W], cur[:, curlo : W - sh]
            )
            cur = dst
            curlo += sh
            if j == 0:
                chain1 = dst  # window-2 sums
        out = self.big()
        # S18[t] = W16[t] + W2[t-16]
        lo = curlo + 16
        nc.vector.tensor_add(out[:, lo:W], cur[:, lo:W], chain1[:, lo - 16 : W - 16])
        for t in tiles:
            self.rel(t)
        return out, lo

    # ---- full pipeline ----
    def build(self):
        nc = self.nc
        with ctile.TileContext(nc) as tc:
            with tc.tile_pool(name="big", bufs=1) as pool, tc.tile_pool(
                name="small", bufs=1
            ) as spool, tc.tile_pool(name="psc", bufs=2, space="PSUM") as pscol, tc.tile_pool(name="psh", bufs=4, space="PSUM") as pshalo, \
                 tc.tile_pool(name="psr", bufs=2, space="PSUM") as psrow:
                self.pool = pool
                self.spool = spool
                self.pscol = pscol
                self.psrow = psrow
                self.pshalo = pshalo
                self.emit()
        nc.finalize()
        return nc

    def mm(self, out, lhsT, rhs):
        self.nc.tensor.matmul(out, lhsT, rhs, start=True, stop=True)

    def emit(self):
        nc = self.nc
        I32 = mybir.dt.int32
        self.eps8 = self.spool.tile([P, 1], F32, tag="c_eps8")
        nc.gpsimd.memset(self.eps8[:, :], 1e-8)
        self.ONESC = self.spool.tile([P, 1], F32, tag="c_ones_col")
        nc.gpsimd.memset(self.ONESC[:, :], 1.0)
        self.nanch = self.spool.tile([P, 1], F32, tag="c_nanch")
        nc.gpsimd.memset(self.nanch[:, :], -self.anchor)
        # PE helper constants: ii[p, m] = m - p; Sh1 = (ii==1); Ident = (ii==0)
        ii = self.spool.tile([P, P], I32, tag="c_iota")
        nc.gpsimd.iota(ii[:, :], pattern=[[1, P]], base=0, channel_multiplier=-1)
        self.Sh1 = self.spool.tile([P, P], F32, tag="c_sh1")
        nc.vector.tensor_single_scalar(
            out=self.Sh1[:, :], in_=ii[:, :], scalar=1, op=Alu.is_equal
        )
        self.Ident = self.spool.tile([P, P], F32, tag="c_ident")
        nc.vector.tensor_single_scalar(
            out=self.Ident[:, :], in_=ii[:, :], scalar=0, op=Alu.is_equal
        )
        # second-order carry matrices for slow alphas (cF > 1e-10)
        self.M2 = {}
        sh2 = None
        for i, a in enumerate(self.alphas):
            cF = (1.0 - a) ** CH
            if cF > 1e-10:
                if sh2 is None:
                    sh2 = self.spool.tile([P, P], F32, tag="c_sh2")
                    nc.vector.tensor_single_scalar(
                        out=sh2[:, :], in_=ii[:, :], scalar=2, op=Alu.is_equal
                    )
                m = self.spool.tile([P, P], F32, tag=f"c_m2_{i}")
                nc.vector.scalar_tensor_tensor(
                    out=m[:, :], in0=sh2[:, :], scalar=float(cF), in1=self.Sh1[:, :],
                    op0=Alu.mult, op1=Alu.add,
                )
                self.M2[i] = m
        # per-conv scan multiplier constants: CONSTS[:, i] = 1 - alpha_i
        self.CONSTS = self.spool.tile([P, len(self.alphas)], F32, tag="c_scanmul")
        for i, a in enumerate(self.alphas):
            nc.gpsimd.memset(self.CONSTS[:, i : i + 1], 1.0 - a)
        self.ones11 = self.spool.tile([1, 1], F32, tag="c_ones11")
        nc.gpsimd.memset(self.ones11[:, :], 1.0)
        self.m50 = self.spool.tile([P, 1], F32, tag="c_m50")
        nc.gpsimd.memset(self.m50[:, :], -50.0)
        self.TG = self.spool.tile([P, CH], I32, tag="c_tg")
        nc.gpsimd.iota(self.TG[:, :], pattern=[[1, CH]], base=0, channel_multiplier=CH)
        Ht = self.load_series(self.DH)
        Lt = self.load_series(self.DL, eng="gpsimd")
        Ct = self.load_series(self.DC)

        # --- KDJ blocks ---
        hchain = self.winchain(Ht, 7, Alu.max)
        lchain = self.winchain(Lt, 7, Alu.min)

        def rsvf(nw):
            hh = self.wincombine(hchain, nw, Alu.max)
            ll = self.wincombine(lchain, nw, Alu.min)
            hl = self.big()
            nc.vector.tensor_sub(hl[:, C0:W], hh[:, C0:W], ll[:, C0:W])
            nc.vector.tensor_scalar_max(hl[:, C0:W], hl[:, C0:W], 1e-8)
            rcp = self.big()
            nc.vector.reciprocal_approx_fast(out=rcp[:, C0:W], in_=hl[:, C0:W])
            num = self.big()
            nc.vector.tensor_sub(num[:, C0:W], Ct[:, C0:W], ll[:, C0:W])
            r0 = self.big()
            nc.vector.tensor_mul(r0[:, C0:W], num[:, C0:W], rcp[:, C0:W])
            self.rel(hh, ll, hl, rcp, num)
            rsv01 = self.big()
            nc.vector.tensor_scalar(
                out=rsv01[:, C0:W], in0=r0[:, C0:W], scalar1=0.0, scalar2=1.0,
                op0=Alu.max, op1=Alu.min,
            )
            self.rel(r0)
            return rsv01

        def jof(Kv, Dv, rows):
            Jv = self.big()
            dkd = self.big()
            self.ptt(dkd[:, 248:W], Kv[:, 248:W], Dv[:, 248:W], Alu.subtract)
            nc.vector.scalar_tensor_tensor(
                out=Jv[:, 248:W], in0=dkd[:, 248:W], scalar=2.0, in1=Kv[:, 248:W],
                op0=Alu.mult, op1=Alu.add,
            )
            self.rel(dkd)
            for ridx, tt_ in zip(rows, (Kv, Dv, Jv)):
                if ridx is not None:
                    self.store_row(ridx, tt_)
            return Jv

        rsv1 = rsvf(204)
        rsv2 = rsvf(18)
        K1, K2 = self.ema_pair(rsv1, 6, rsv2, 8, 100.0, 100.0)
        self.rel(rsv1, rsv2)
        D1, D2 = self.ema_pair(K1, 7, K2, 9)
        J1 = jof(K1, D1, (9, 10, 11))
        J2 = jof(K2, D2, (12, 13, 14))
        self.rel(K1, D1, K2, D2)
        # --- TEMA3 + TEMAP2 chains (stage-interleaved ema pairs) ---
        EMA1, E21 = self.ema_pair(Ct, 0, Ct, 3)
        EMA2, E221 = self.ema_pair(EMA1, 1, E21, 4)
        EMA3, E231 = self.ema_pair(EMA2, 2, E221, 5)
        TEMA3 = self.big()
        d = self.big()
        self.ptt(d[:, 248:W], EMA1[:, 248:W], EMA2[:, 248:W], Alu.subtract)
        nc.vector.scalar_tensor_tensor(
            out=TEMA3[:, 248:W], in0=d[:, 248:W], scalar=3.0, in1=EMA3[:, 248:W],
            op0=Alu.mult, op1=Alu.add,
        )
        self.rel(EMA1, EMA2, EMA3, d)
        self.store_row(4, TEMA3)
        TEMAP2 = self.big()
        d = self.big()
        self.ptt(d[:, 248:W], E21[:, 248:W], E221[:, 248:W], Alu.subtract)
        nc.vector.scalar_tensor_tensor(
            out=TEMAP2[:, 248:W], in0=d[:, 248:W], scalar=3.0, in1=E231[:, 248:W],
            op0=Alu.mult, op1=Alu.add,
        )
        self.rel(E21, E221, E231, d)

        # --- stdp(C,18) via per-partition-anchored rolling cumsums ---
        # anchor per partition (C at col 232) kills catastrophic cancellation
        npc = self.small()
        nc.scalar.mul(npc[:, 0:1], Ct[:, 232:233], -1.0)
        dev = self.big()
        nc.scalar.activation(dev[:, 232:W], Ct[:, 232:W], Act.Identity, bias=npc[:, 0:1])
        dev2 = self.big()
        nc.scalar.activation(dev2[:, 232:W], Ct[:, 232:W], Act.Square, bias=npc[:, 0:1])
        NS = W - 232
        ones_bc = self.ONESC[:, 0:1].broadcast_to([P, NS])
        cs = self.big()
        nc.vector.tensor_tensor_scan(
            out=cs[:, 232:W], data0=ones_bc, data1=dev[:, 232:W],
            initial=0.0, op0=Alu.mult, op1=Alu.add,
        )
        cs2 = self.big()
        nc.vector.tensor_tensor_scan(
            out=cs2[:, 232:W], data0=ones_bc, data1=dev2[:, 232:W],
            initial=0.0, op0=Alu.mult, op1=Alu.add,
        )
        self.rel(dev, dev2)
        m = self.big()
        nc.vector.scalar_tensor_tensor(
            out=m[:, 250:W], in0=cs[:, 232 : W - 18], scalar=-1.0,
            in1=cs[:, 250:W], op0=Alu.mult, op1=Alu.add,
        )
        nc.scalar.mul(m[:, 250:W], m[:, 250:W], 1.0 / 18.0)
        ex2 = self.big()
        nc.vector.scalar_tensor_tensor(
            out=ex2[:, 250:W], in0=cs2[:, 232 : W - 18], scalar=-1.0,
            in1=cs2[:, 250:W], op0=Alu.mult, op1=Alu.add,
        )
        nc.scalar.mul(ex2[:, 250:W], ex2[:, 250:W], 1.0 / 18.0)
        self.rel(cs, cs2)
        mm = self.big()
        nc.scalar.square(mm[:, 250:W], m[:, 250:W])
        var = self.big()
        self.ptt(var[:, 250:W], ex2[:, 250:W], mm[:, 250:W], Alu.subtract)
        varc = self.big()
        nc.vector.tensor_scalar_max(varc[:, 250:W], var[:, 250:W], 0.0)
        DIS = self.big()
        nc.scalar.activation(DIS[:, 250:W], varc[:, 250:W], Act.Sqrt)
        self.rel(m, ex2, mm, var, varc)
        TEU3 = self.big()
        self.ptt(TEU3[:, C0:W], TEMA3[:, C0:W], DIS[:, C0:W], Alu.add)
        TED = self.big()
        self.ptt(TED[:, C0:W], TEMA3[:, C0:W], DIS[:, C0:W], Alu.subtract)
        self.store_row(3, TEU3)
        self.store_row(5, TED)
        self.rel(DIS, TEU3, TED)

        rsv3 = rsvf(9)
        rsvn = rsvf(36)
        K3, KN3 = self.ema_pair(rsv3, 10, rsvn, 12, 100.0, 100.0)
        self.rel(rsv3, rsvn)
        D3, DN3x = self.ema_pair(K3, 11, KN3, 13, 1.0, 2.0)
        J3 = jof(K3, D3, (15, 16, 17))
        JN3 = self.big()
        nc.vector.scalar_tensor_tensor(
            out=JN3[:, 248:W], in0=KN3[:, 248:W], scalar=3.0,
            in1=DN3x[:, 248:W], op0=Alu.mult, op1=Alu.subtract,
        )
        self.store_row(18, JN3)
        self.rel(K3, D3, KN3, DN3x, JN3)
        for ch_ in (hchain, lchain):
            for kk, (tt_, _) in ch_.items():
                if kk > 1:
                    self.rel(tt_)
        self.rel(Ht, Lt)



        # --- T ratios ---
        def tdiff(xt, lag, row_idx, lo=254):
            dt_ = self.big()
            nc.vector.tensor_tensor(
                out=dt_[:, lo:W], in0=xt[:, lo:W], in1=xt[:, lo - lag : W - lag],
                op=Alu.subtract,
            )
            ab = self.big()
            nc.scalar.activation(ab[:, lo:W], xt[:, lo - lag : W - lag], Act.Abs)
            abe = self.big()
            nc.scalar.activation(abe[:, lo:W], ab[:, lo:W], Act.Identity, bias=self.eps8[:, 0:1])
            rr = self.big()
            nc.vector.reciprocal_approx_fast(out=rr[:, lo:W], in_=abe[:, lo:W])
            ts_ = self.big()
            nc.vector.tensor_mul(ts_[:, lo:W], dt_[:, lo:W], rr[:, lo:W])
            self.rel(dt_, ab, abe, rr)
            if row_idx is not None:
                self.store_row(row_idx, ts_)
            return ts_

        T3s = tdiff(TEMA3, 6, 8)
        T1s = tdiff(TEMA3, 1, 6)
        T2s = tdiff(TEMAP2, 6, 7)
        self.rel(TEMAP2, TEMA3)

        # --- JX family ---
        JXb = self.big()
        u = self.big()
        nc.vector.tensor_mul(u[:, 254:W], J3[:, 254:W], T1s[:, 254:W])
        v = self.big()
        self.ptt(v[:, 254:W], J1[:, 254:W], J2[:, 254:W], Alu.add)
        self.ptt(JXb[:, 254:W], u[:, 254:W], v[:, 254:W], Alu.add)
        self.rel(u, v, J3, T1s)
        F1 = self.big()
        nc.vector.tensor_mul(F1[:, 254:W], J2[:, 254:W], T3s[:, 254:W])
        self.rel(J2, T3s)
        F2 = self.big()
        self.ptt(F2[:, 254:W], J1[:, 254:W], T2s[:, 254:W], Alu.mult)
        self.rel(J1, T2s)
        self.store_row(19, JXb)
        self.store_row(20, F1)
        self.store_row(21, F2)

        EMA_JX, EMA_F1 = self.ema_pair(JXb, 14, F1, 15)
        EMA_F2, EMA8_JX = self.ema_pair(F2, 16, JXb, 17)
        EMA8_F1, EMA8_F2 = self.ema_pair(F1, 18, F2, 19)
        self.store_row(22, EMA_JX)
        self.store_row(23, EMA_F1)
        self.store_row(24, EMA_F2)

        def jx_combine(base, f1, f2, row_idx, lo=254):
            w_ = self.big()
            self.ptt(w_[:, lo:W], f1[:, lo:W], f2[:, lo:W], Alu.add)
            z = self.big()
            nc.vector.scalar_tensor_tensor(
                out=z[:, lo:W], in0=w_[:, lo:W], scalar=6.0, in1=base[:, lo:W],
                op0=Alu.mult, op1=Alu.add,
            )
            out = self.big()
            nc.scalar.activation(out[:, lo:W], z[:, lo:W], Act.Identity, bias=self.m50[:, 0:1])
            self.rel(w_, z)
            self.store_row(row_idx, out)
            return out

        # need col 255 for the cross lag -> compute from col 254
        JX = jx_combine(JXb, F1, F2, 27, lo=254)
        EMAJX = jx_combine(EMA_JX, EMA_F1, EMA_F2, 28, lo=254)
        EMAJX8 = jx_combine(EMA8_JX, EMA8_F1, EMA8_F2, 29, lo=254)
        self.rel(JXb, F1, F2, EMA_JX, EMA_F1, EMA_F2, EMA8_JX, EMA8_F1, EMA8_F2)
        self.rel(EMAJX8)

        # --- crosses + segmented MA scans ---
        def macond(updown):
            g = self.big()
            l = self.big()
            if updown == "up":
                nc.vector.tensor_tensor(
                    out=g[:, 255:W], in0=JX[:, 255:W], in1=EMAJX[:, 255:W],
                    op=Alu.is_gt,
                )
                nc.vector.tensor_tensor(
                    out=l[:, 255:W], in0=JX[:, 254 : W - 1],
                    in1=EMAJX[:, 254 : W - 1], op=Alu.is_le,
                )
            else:
                nc.vector.tensor_tensor(
                    out=g[:, 255:W], in0=JX[:, 255:W], in1=EMAJX[:, 255:W],
                    op=Alu.is_lt,
                )
                nc.vector.tensor_tensor(
                    out=l[:, 255:W], in0=JX[:, 254 : W - 1],
                    in1=EMAJX[:, 254 : W - 1], op=Alu.is_ge,
                )
            cond = self.big()
            self.ptt(cond[:, 255:W], g[:, 255:W], l[:, 255:W], Alu.mult)
            m_ = self.big()
            self.pts(m_[:, 255:W], cond[:, 255:W], -1.0, 1.0, Alu.mult, Alu.add)
            self.rel(g, l)
            return cond, m_

        # Manual expansion (cnt first to derive A, then S and seen share it)
        for updown, row_idx, diag_idx in (("dn", 25, 1), ("up", 26, 0)):
            cond, m_ = macond(updown)
            dmask = self.big()
            self.ptt(dmask[:, C0:W], Ct[:, C0:W], m_[:, C0:W], Alu.mult)

            # --- cnt: scan1, A row via PE transpose, affine chain, scan2 ---
            cnt_s = self.big()
            nc.vector.tensor_tensor_scan(
                out=cnt_s[:, C0:W], data0=m_[:, C0:W], data1=m_[:, C0:W],
                initial=0.0, op0=Alu.mult, op1=Alu.add,
            )
            acol = self.small()
            nc.vector.tensor_single_scalar(
                out=acol[:, 0:1], in_=cnt_s[:, W - 1 : W], scalar=float(CH),
                op=Alu.is_ge,
            )
            par = self.psrow.tile([1, P], F32, tag="psrow")
            self.mm(par[0:1, 0:P], acol[:, 0:1], self.Ident[:, :])
            arow = self.row()
            nc.vector.tensor_copy(arow[0:1, 0:P], par[0:1, 0:P])
            self.rels(acol)

            def chain(scan1_tile, op0, op1, d0, d1):
                """affine partition chain for a scan1 result, then scan2
                in place (initial from PSUM via PE shift-back)."""
                rowt = self.row129()
                nc.gpsimd.memset(rowt[0:1, 0:1], 0.0)
                pbr = self.psrow.tile([1, P], F32, tag="psrow")
                self.mm(pbr[0:1, 0:P], scan1_tile[:, W - 1 : W], self.Ident[:, :])
                nc.vector.tensor_tensor_scan(
                    out=rowt[0:1, 1 : P + 1], data0=arow[0:1, 0:P],
                    data1=pbr[0:1, 0:P], initial=0.0, op0=Alu.mult, op1=Alu.add,
                )
                pcc = self.pscol.tile([P, 1], F32, tag="pscol")
                self.mm(pcc[:, 0:1], rowt[0:1, 0:P], self.ones11[0:1, 0:1])
                nc.vector.tensor_tensor_scan(
                    out=scan1_tile[:, C0:W], data0=d0, data1=d1,
                    initial=pcc[:, 0:1], op0=op0, op1=op1,
                )
                self.relr129(rowt)

            chain(cnt_s, Alu.mult, Alu.add, m_[:, C0:W], m_[:, C0:W])

            # --- S: sum since last event ---
            Ssum = self.big()
            nc.vector.tensor_tensor_scan(
                out=Ssum[:, C0:W], data0=m_[:, C0:W], data1=dmask[:, C0:W],
                initial=0.0, op0=Alu.mult, op1=Alu.add,
            )
            chain(Ssum, Alu.mult, Alu.add, m_[:, C0:W], dmask[:, C0:W])

            # --- seen: cnt counts from ext start when no event has occurred,
            # so seen[t] = (cnt[t] <= ext_index[t]) ---
            seen = self.big()
            dcol = self.small()
            nc.vector.scalar_tensor_tensor(
                out=seen[:, C0:W], in0=cnt_s[:, C0:W], scalar=1.0,
                in1=self.TG[:, 0:CH], op0=Alu.mult, op1=Alu.is_le,
                accum_out=dcol[:, 0:1],
            )
            self.relr(arow)
            self.rel(cond, m_, dmask)

            # ma = (S * recip(max(cnt,1))) * seen
            rc = self.big()
            nc.vector.tensor_scalar_max(rc[:, C0:W], cnt_s[:, C0:W], 1.0)
            rcp = self.big()
            nc.vector.reciprocal_approx_fast(out=rcp[:, C0:W], in_=rc[:, C0:W])
            ma0 = self.big()
            self.ptt(ma0[:, C0:W], Ssum[:, C0:W], rcp[:, C0:W], Alu.mult)
            ma = self.big()
            nc.vector.tensor_mul(ma[:, C0:W], ma0[:, C0:W], seen[:, C0:W])
            self.rel(rc, rcp, ma0, cnt_s, Ssum)
            self.store_row(row_idx, ma)

            # diag: min of seen over partitions 5..127 (covers valid region)
            drow = self.row()
            nc.sync.dma_start(out=drow[0:1, 0 : P - 1], in_=dcol[1:P, 0:1])
            done = self.spool.tile([1, 1], F32, tag=f"diag{diag_idx}")
            nc.vector.tensor_reduce(
                out=done[0:1, 0:1], in_=drow[0:1, 0 : P - 1],
                axis=mybir.AxisListType.X, op=Alu.min,
            )
            self.relr(drow)
            nc.sync.dma_start(
                out=self.DIAG[diag_idx : diag_idx + 1].rearrange(
                    "(a b) -> a b", a=1, b=1
                ),
                in_=done[0:1, 0:1],
            )
            self.rels(dcol)
            self.rel(seen, ma)

        self.rel(Ct, JX, EMAJX)


_CACHE = {}


def _build(alphas, anchor):
    key = (tuple(round(float(a), 12) for a in alphas), round(float(anchor), 6))
    if key not in _CACHE:
        kb = KB(alphas, anchor)
        _CACHE[key] = kb.build()
    return _CACHE[key]


def _shard(x):
    """per-core input arrays [DLEN], clamp-padded on the global left."""
    outs = []
    for mcore in range(NCORES):
        lo = (mcore + 1) * S - DLEN
        if lo < 0:
            d = np.concatenate(
                [np.full(-lo, x[0], np.float32), x[0 : (mcore + 1) * S]]
            )
        else:
            d = x[lo : (mcore + 1) * S]
        outs.append(np.ascontiguousarray(d, np.float32))
    return outs


def _host_ma(C, JX, EJ):
    """exact host fallback for ma rows (numpy, global)."""
    f32 = np.float32
    T_ = len(C)
    lag = lambda x: np.concatenate([x[:1], x[:-1]])
    JXp, EJp = lag(JX), lag(EJ)
    res = {}
    cs = np.concatenate([[0.0], np.cumsum(C.astype(np.float64))])
    t_idx = np.arange(T_)
    for key, cond in (
        ("dn", (JX < EJ) & (JXp >= EJp)),
        ("up", (JX > EJ) & (JXp <= EJp)),
    ):
        last = np.maximum.accumulate(np.where(cond, t_idx, -1))
        csl = cs[np.maximum(last, 0) + 1]
        s = cs[t_idx + 1] - csl
        n = t_idx - last
        res[key] = np.where(
            (last >= 0) & (n > 0), s / np.maximum(n, 1), 0.0
        ).astype(f32)
    return res["dn"], res["up"]


def run_cores(inputs, trace=False):
    """compile (cached) + run on 8 cores; returns (results, BassKernelResults)."""
    C = np.ascontiguousarray(inputs["C"], np.float32)
    H = np.ascontiguousarray(inputs["H"], np.float32)
    L = np.ascontiguousarray(inputs["L"], np.float32)
    w = np.asarray(inputs["w_alphas"], np.float32)
    alphas = [float(1.0 / (1.0 + math.exp(-float(x)))) for x in w]
    nc = _build(alphas, float(C[0]))
    dc, dh, dl = _shard(C), _shard(H), _shard(L)
    in_maps = [
        {"DC": dc[m], "DH": dh[m], "DL": dl[m]} for m in range(NCORES)
    ]
    res = run_bass_kernel_spmd(
        nc, in_maps, core_ids=list(range(NCORES)), trace=trace
    )
    return res


def kernel(C, H, L, w_alphas):
    inputs = {"C": C, "H": H, "L": L, "w_alphas": w_alphas}
    res = run_cores(inputs)
    outs = [res.results[m]["OUT"].reshape(NROWS, EXT)[:, HALO:] for m in range(NCORES)]
    full = np.concatenate(outs, axis=1)
    full[0] = np.asarray(C, np.float32)
    full[1] = np.asarray(H, np.float32)
    full[2] = np.asarray(L, np.float32)

    # host patch: reference's partial-window std for the first 17 bars
    Cg = np.asarray(C, np.float64)[:17]
    for t in range(17):
        wdw = Cg[: t + 1]
        dis = math.sqrt(max(np.mean(wdw * wdw) - np.mean(wdw) ** 2, 0.0))
        full[3, t] = np.float32(full[4, t] + dis)
        full[5, t] = np.float32(full[4, t] - dis)

    # diag check: cross gap exceeded the halo on some core -> exact host fix
    need_fix = False
    for mcore in range(1, NCORES):
        dg = res.results[mcore]["DIAG"]
        if dg.min() < CH - 0.5:
            need_fix = True
    if need_fix:
        ma_dn, ma_up = _host_ma(
            np.asarray(C, np.float32), full[27], full[28]
        )
        full[25] = ma_dn
        full[26] = ma_up
    return full.astype(np.float32)

